# revision 5
# baseline (speedup 1.0000x reference)
"""Multi-head attention forward on 8 Trainium2 NeuronCores.

For x [16, 1024, 512], w_qkv [512, 1536], w_out [512, 512], b_out [512]:
    qkv = x @ w_qkv; q, k, v = split(qkv)
    out = softmax(q k^T / sqrt(512)) v          (8 heads, head_dim 64)
    return out @ w_out + b_out                  [16, 1024, 512]

Sharding: data-parallel over batch — 2 batches per core, no collectives.

Per-core design (~195us; PE-bound with the ACT exp stream just behind):
  - All PE operands bf16 (1 cycle/row like f32r but half the DMA/SBUF);
    PSUM accumulation fp32. Head pairs run concurrently in the two 64-row
    PE groups (scores K=64, base partitions 0/64).
  - Scores are computed transposed per 128-key j-tile; softmax skips the
    max-subtraction (scaled scores stay in ~[-2.5, 2.5]); one [128, 1024]
    exp per j-step covers both heads (ACT is the 142us secondary bound).
  - Softmax denominators come from the attn@v matmul itself: stationary
    [ones x64 | v_h] (M=128) puts the denominator broadcast across
    partitions 0-63 and ctx in 64-127 of the same PSUM bank. The norm is
    one full-tile eviction (releases the accumulator), an aligned
    reciprocal_approx_fast, one cross-partition hop, two multiplies; the
    head pair's cT hemispheres are swapped (w_out rows permuted on host).
  - Emission is software-pipelined with lookahead-2 scores and a STATIC
    per-step filler schedule: every projection / out-projection matmul
    group is placed just-in-time so per-step PE load tracks the ACT exp
    cadence in both batch regions, the PE never head-blocks, and the HAM
    clock gate stays at 2.4 GHz; 8 out-proj groups are reserved for the
    drain. Startup DMAs are split by first consumer across the sync and
    ACT dispatchers.
"""

import numpy as np

import concourse.bass as bass
from concourse import bacc
import concourse.mybir as mybir
import concourse.tile as tile
from concourse.bass_utils import run_bass_kernel_spmd

F32 = mybir.dt.float32
BF16 = mybir.dt.bfloat16

N_CORES = 8
B = 16                 # global batch
BC = B // N_CORES      # batches per core
SEQ = 1024
TOK = BC * SEQ         # tokens per core
D = 512                # model dim
H = 8                  # heads
DH = D // H            # head dim = 64
SCALE = float(D) ** -0.5

PHASES = 3             # debug: 1=qkv proj only, 2=+attention, 3=full
REPEAT = 1
P = 128
KO = D // P            # 4 contraction chunks of 128
NT = TOK // 512        # 4 moving 512-token slabs
MT = TOK // P          # 16 token tiles of 128
JT = SEQ // P          # 8 key tiles per batch
MO_ORDER = [0, 4, 1, 5, 2, 6, 3, 7]  # q/k interleave so pair m ready early


def _build_program():
    nc = bacc.Bacc("TRN2", target_bir_lowering=False, debug=False)

    x_d = nc.dram_tensor("xT", [D, TOK], BF16, kind="ExternalInput")
    wqkv_d = nc.dram_tensor("w_qkv", [D, 3 * D], BF16, kind="ExternalInput")
    wout_d = nc.dram_tensor("w_out", [D, D], BF16, kind="ExternalInput")
    bout_d = nc.dram_tensor("b_out", [D], F32, kind="ExternalInput")
    out_d = nc.dram_tensor("out", [TOK, D], F32, kind="ExternalOutput")

    with tile.TileContext(nc) as tc:
        for _rep in range(REPEAT):
            _emit(tc, x_d.ap(), wqkv_d.ap(), wout_d.ap(), bout_d.ap(), out_d.ap())
    nc.compile()
    return nc


def _emit(tc, x_d, wqkv_d, wout_d, bout_d, out_d):
    nc = tc.nc
    Exp = mybir.ActivationFunctionType.Exp
    mult = mybir.AluOpType.mult
    add = mybir.AluOpType.add

    from contextlib import ExitStack
    with ExitStack() as ctx:
        persist = ctx.enter_context(tc.tile_pool(name="persist", bufs=1))

        # ---- persistent SBUF ----
        w_qkv_sb = persist.tile([P, KO, 3 * D], BF16)
        xT = persist.tile([P, KO, TOK], BF16)
        qkT = persist.tile([P, H, TOK], BF16)          # do-blocks: q 0-3 / k 4-7 interleaved per MO_ORDER semantics
        v_ext = persist.tile([P, BC * JT, H, 2 * DH], BF16)  # [j, jt, h, v|ones]
        cT = persist.tile([P, KO, TOK], BF16)
        w_out_sb = persist.tile([P, KO, D], BF16)
        b_bc = persist.tile([P, D], F32)

        # ---- prologue DMAs. Early set (startup critical path) dispatches
        # immediately; the bulk set sits behind a blocker DMA that depends
        # on the first v eviction, so the SP FIFO delays its dispatch and
        # the early set gets the full fabric bandwidth. ----
        wq_r = wqkv_d.rearrange("(ko p) n -> p ko n", p=P)
        x_r = x_d.rearrange("(c p) t -> p c t", p=P)
        # early set on sync, in first-consumer order; xT b0 and the w_v
        # block are split so each prologue/ramp group's semaphore fires as
        # soon as ITS slice of the transfer lands
        # early set on sync, in first-consumer order; xT b0 and the w_v
        # block are split so each prologue/ramp group's semaphore fires as
        # soon as ITS slice of the transfer lands
        nc.sync.dma_start(out=xT[:, :, 0:512], in_=x_r[:, :, 0:512])
        for mo in (4, 0):
            nc.sync.dma_start(
                out=w_qkv_sb[:, :, mo * P : (mo + 1) * P],
                in_=wq_r[:, :, mo * P : (mo + 1) * P],
            )
        nc.sync.dma_start(
            out=w_qkv_sb[:, :, 2 * D : 2 * D + P], in_=wq_r[:, :, 2 * D : 2 * D + P]
        )
        nc.sync.dma_start(out=xT[:, :, 512:SEQ], in_=x_r[:, :, 512:SEQ])
        for mo in (1, 5):
            nc.sync.dma_start(
                out=w_qkv_sb[:, :, mo * P : (mo + 1) * P],
                in_=wq_r[:, :, mo * P : (mo + 1) * P],
            )
        nc.sync.dma_start(
            out=w_qkv_sb[:, :, 2 * D + P : 3 * D], in_=wq_r[:, :, 2 * D + P : 3 * D]
        )
        # bulk set dispatched from the (otherwise idle) ACT sequencer: its
        # descriptor generation is off the sync engine's critical path and
        # starts a few us later, giving the early set the fabric first
        for mo in (2, 6, 3, 7):
            nc.scalar.dma_start(
                out=w_qkv_sb[:, :, mo * P : (mo + 1) * P],
                in_=wq_r[:, :, mo * P : (mo + 1) * P],
            )
        nc.scalar.dma_start(out=xT[:, :, SEQ:TOK], in_=x_r[:, :, SEQ:TOK])
        nc.scalar.dma_start(
            out=w_out_sb, in_=wout_d.rearrange("(ko p) n -> p ko n", p=P)
        )
        nc.scalar.dma_start(out=b_bc, in_=bout_d.unsqueeze(0).to_broadcast((P, D)))

        # ones half of v_ext (idle gpsimd, off critical path); layout is
        # [ones | v] so softmax denominators land in partitions 0-63
        nc.gpsimd.memset(v_ext[:, :, :, 0:DH], 1.0)

        ps_mm = ctx.enter_context(tc.tile_pool(name="ps_mm", bufs=2, space="PSUM"))

        # HAM warm-up: ~3.4us of dependency-free matmuls on a memset scratch
        # tile, issued while the startup DMAs are still in flight, so the
        # clock gate is already at 2.4 GHz when the real prologue runs.
        warm_sb = persist.tile([P, 512], BF16)
        nc.vector.memset(warm_sb, 1.0)
        warm_ps = ps_mm.tile([P, D], F32, tag="fps", name="warm_ps")
        for _ in range(8):
            nc.tensor.matmul(warm_ps, warm_sb[:, 0:P], warm_sb)
        dve = nc.vector

        # ---- projection helpers (emit one (kind, idx) group: 4 MMs + evict) ----
        def emit_v_span(t, dlo, dhi):  # token tile t, v-dim range [dlo, dhi)
            ps = ps_mm.tile([P, D], F32, tag="fps", name="vps")
            for ko in range(KO):
                nc.tensor.matmul(
                    ps[:, dlo:dhi],
                    xT[:, ko, t * P : (t + 1) * P],
                    w_qkv_sb[:, ko, 2 * D + dlo : 2 * D + dhi],
                    start=(ko == 0),
                    stop=(ko == KO - 1),
                )
            dve.tensor_copy(
                v_ext[:, t, dlo // DH : dhi // DH, DH : 2 * DH],
                ps[:, dlo:dhi].rearrange("p (h d) -> p h d", d=DH),
            )

        def emit_v_group(t):  # t: global token tile 0..15
            emit_v_span(t, 0, D)

        def emit_qk_span(mo, lo, hi):  # token span [lo, hi)
            ps = ps_mm.tile([P, D], F32, tag="fps", name="qkps")
            for ko in range(KO):
                nc.tensor.matmul(
                    ps[:, 0 : hi - lo],
                    w_qkv_sb[:, ko, mo * P : (mo + 1) * P],
                    xT[:, ko, lo:hi],
                    start=(ko == 0),
                    stop=(ko == KO - 1),
                )
            dve.tensor_copy(qkT[:, mo, lo:hi], ps[:, 0 : hi - lo])

        def emit_qk_group(mo, nt):  # nt: global 512-token slab 0..3
            emit_qk_span(mo, nt * 512, (nt + 1) * 512)

        def emit_out_group(it):  # it: global token tile 0..15
            f_ps = ps_mm.tile([P, D], F32, tag="fps", name="fps")
            for ko in range(KO):
                nc.tensor.matmul(
                    f_ps,
                    cT[:, ko, it * P : (it + 1) * P],
                    w_out_sb[:, ko, :],
                    start=(ko == 0),
                    stop=(ko == KO - 1),
                )
            o_sb = osb_pool.tile([P, D], F32, tag="o_sb", name="o_sb")
            dve.tensor_tensor(o_sb, f_ps, b_bc, add)
            nc.sync.dma_start(out=out_grp[it], in_=o_sb)

        out_grp = out_d.rearrange("(t p) d -> t p d", p=P)

        # startup projections: minimum for the first scores tile. The first
        # k-projection covers only j-tiles 0-1 (tokens 0-256) so the first
        # exp starts as early as possible; v0/v1 + the rest go to sched[0-1]
        # (before attnv(0) in the PE FIFO, after the bootstrap scores).
        prologue_groups = [
            ("qks", 4, 0, 256),
            ("qk", 0, 0),
        ]
        # Filler work sits on a STATIC per-step schedule: each prep group as
        # late as its consumer allows, spread ~1 group per 3 steps, so the
        # per-step PE load stays at the ACT exp cadence in both batch
        # regions (reactive pacing front-loads batch 0 and starves the PE
        # in batch 1). Out-proj is spread over late batch-1 steps; the last
        # 8 out groups are reserved for the drain so the PE stays busy (and
        # the HAM clock warm) through the final norm chain.
        sched = {}

        def _at(si, g):
            sched.setdefault(si, []).append(g)

        # block-0 ramp: v arrives head-pair-sliced (tiny cold groups) just
        # ahead of each attnv; q/k blocks for m1 interleave at every other
        # step; the rest of v (heads 2-7) lands at steps 8-15.
        _at(0, ("qks", 4, 256, 512))
        for t in range(JT):
            _at(t, ("vs", t, 0, 2 * DH))
        _at(1, ("qk", 4, 1))
        _at(2, ("qk", 0, 1))
        for k, g in enumerate([("qk", mo, nt) for mo in (1, 5) for nt in (0, 1)]):
            _at((3, 5, 7, 9)[k], g)
        for t in range(JT):
            _at(8 + t, ("vs", t, 2 * DH, D))
        for k, g in enumerate([("qk", mo, nt) for mo in (2, 6) for nt in (0, 1)]):
            _at(16 + 2 * k, g)
        for k, g in enumerate([("qk", mo, nt) for mo in (3, 7) for nt in (0, 1)]):
            _at(24 + 2 * k, g)
        b1_prep = (
            [("qk", mo, nt) for mo in (0, 4) for nt in (2, 3)]
            + [("v", t) for t in range(JT, 2 * JT)]
            + [("qk", mo, nt) for mo in (1, 5, 2, 6, 3, 7) for nt in (2, 3)]
        )
        for k, g in enumerate(b1_prep):
            _at(32 + 3 * k, g)
        for k in range(JT):
            _at(103 + 3 * k, ("out", k))
        drain_groups = [("out", it) for it in range(JT, MT)]

        def emit_group(g):
            if g[0] == "v":
                emit_v_group(g[1])
            elif g[0] == "qk":
                emit_qk_group(g[1], g[2])
            elif g[0] == "qks":
                emit_qk_span(g[1], g[2], g[3])
            elif g[0] == "vs":
                emit_v_span(g[1], g[2], g[3])
            else:
                emit_out_group(g[1])

        def emit_sched(si):
            for g in sched.pop(si, ()):
                emit_group(g)

        osb_pool = ctx.enter_context(tc.tile_pool(name="osb", bufs=3))

        for g in prologue_groups:
            emit_group(g)

        if PHASES < 2:
            # drain remaining projections, dump v for debug
            for si in sorted(sched):
                for g in sched[si]:
                    if g[0] != "out":
                        emit_group(g)
            for t in range(MT):
                nc.sync.dma_start(out=out_grp[t], in_=v_ext[:, t, :, DH : 2 * DH])
            return

        # ---- attention ----
        ps_T = ctx.enter_context(tc.tile_pool(name="ps_T", bufs=2, space="PSUM"))
        ps_o = ctx.enter_context(tc.tile_pool(name="ps_o", bufs=1, space="PSUM"))
        p_pool = ctx.enter_context(tc.tile_pool(name="p_pool", bufs=3))
        n_pool = ctx.enter_context(tc.tile_pool(name="n_pool", bufs=2))

        def kT(h, b, jt):
            lo = DH * (h % 2)
            return qkT[lo : lo + DH, 4 + h // 2, b * SEQ + jt * P : b * SEQ + (jt + 1) * P]

        def qT(h, b, ih):
            lo = DH * (h % 2)
            return qkT[lo : lo + DH, h // 2, b * SEQ + ih * 512 : b * SEQ + (ih + 1) * 512]

        def emit_scores(b, m, ih, jt):
            h1, h2 = 2 * m, 2 * m + 1
            T = ps_T.tile([P, 2, 512], F32, tag="T", name="T")
            nc.tensor.matmul(T[:, 0, :], kT(h1, b, jt), qT(h1, b, ih))
            nc.tensor.matmul(T[:, 1, :], kT(h2, b, jt), qT(h2, b, ih))
            return T

        blocks = [
            (b, m, ih) for b in range(BC) for m in range(H // 2) for ih in range(2)
        ]
        steps = [(blk, jt) for blk in blocks for jt in range(JT)]

        # software pipeline, lookahead 2: scores(si+2) are emitted at step
        # si AFTER attnv(si) — both wait on exp(si) (RAW for attnv, T-buffer
        # WAR for scores with bufs=2), so the ACT exp stream never starves
        # and the only never-waiting PE work (fillers) goes first in the
        # step to absorb the exp latency.
        T_tiles = {}
        outABs = {}
        T_tiles[0] = emit_scores(*steps[0][0], steps[0][1])
        T_tiles[1] = emit_scores(*steps[1][0], steps[1][1])
        for si, (blk, jt) in enumerate(steps):
            b, m, ih = blk
            h1, h2 = 2 * m, 2 * m + 1
            if jt == 0:
                outABs[blk] = ps_o.tile([P, 2, 512], F32, tag="oAB", name="oAB")
            # exp of this step
            T = T_tiles.pop(si)
            pT = p_pool.tile([P, 2, 512], BF16, tag="pT", name="pT")
            nc.scalar.activation(pT, T, Exp, scale=SCALE)
            # scheduled filler keeps PE busy while exp completes
            emit_sched(si)
            # attn@v (+denominator broadcast via ones columns)
            outAB = outABs[blk]
            jg = b * JT + jt
            nc.tensor.matmul(
                outAB[:, 0, :], v_ext[:, jg, h1, :], pT[:, 0, :],
                start=(jt == 0), stop=(jt == JT - 1),
            )
            nc.tensor.matmul(
                outAB[:, 1, :], v_ext[:, jg, h2, :], pT[:, 1, :],
                start=(jt == 0), stop=(jt == JT - 1),
            )
            # scores two steps ahead
            if si + 2 < len(steps):
                nblk, njt = steps[si + 2]
                T_tiles[si + 2] = emit_scores(*nblk, njt)
            # normalization at block end: one full-tile eviction releases the
            # PSUM accumulator immediately. Denominators are in partitions
            # 0-63 ([ones | v] stationary), ctx in 64-127. recip runs
            # partition-aligned on the low half (required by the custom DVE
            # op); the idle gpsimd hops the reciprocals to the high half so
            # both multiplies read a single quadrant-pair. Head h1 lands in
            # cT rows 64-127, h2 in rows 0-63 (w_out rows permuted on host).
            if jt == JT - 1:
                # per-bank chains: each bank's evict releases its half of the
                # accumulator immediately and its mult lands ~1us sooner
                ctxf = n_pool.tile([P, 2, 512], F32, tag="ctxf", name="ctxf")
                rr = n_pool.tile([P, 2, 512], F32, tag="rr", name="rr")
                cols = slice(b * SEQ + ih * 512, b * SEQ + (ih + 1) * 512)
                rows = (slice(64, P), slice(0, 64))  # h1 -> cT hi, h2 -> lo
                for u in range(2):
                    dve.tensor_copy(ctxf[:, u, :], outAB[:, u, :])
                    dve.reciprocal_approx_fast(
                        out=rr[0:64, u, :], in_=ctxf[0:64, u, :]
                    )
                    dve.tensor_copy(rr[64:P, u, :], rr[0:64, u, :])
                    dve.tensor_tensor(
                        cT[rows[u], m, cols], ctxf[64:P, u, :], rr[64:P, u, :], mult
                    )
                del outABs[blk]

        if PHASES < 3:
            nc.sync.dma_start(
                out=out_d.rearrange("(t p) d -> p t d", p=P),
                in_=cT.rearrange("p a (c d) -> p (a c) d", d=D),
            )
            return

        # drain: reserved out-proj groups (it8-11 ready immediately — they
        # cover the final norm chain; it12-15 wait only on the last mults)
        for si in sorted(sched):
            for g in sched[si]:
                emit_group(g)
        for g in drain_groups:
            emit_group(g)


_CACHE = {}


def _get_nc():
    key = (PHASES, REPEAT)
    if key not in _CACHE:
        _CACHE[key] = _build_program()
    return _CACHE[key]


def _bf16(a):
    import ml_dtypes

    return np.ascontiguousarray(a, dtype=np.float32).astype(ml_dtypes.bfloat16)


_WOUT_PERM = np.concatenate(
    [
        np.arange(DH) + DH * (2 * m + (1 - half))
        for m in range(H // 2)
        for half in range(2)
    ]
)  # chunk m rows = [h2 dims | h1 dims]


def make_in_maps(inputs):
    x = np.asarray(inputs["x"], dtype=np.float32)
    w_qkv = _bf16(inputs["w_qkv"])
    w_out = _bf16(np.asarray(inputs["w_out"], dtype=np.float32)[_WOUT_PERM, :])
    b_out = np.ascontiguousarray(np.asarray(inputs["b_out"], dtype=np.float32))
    return [
        {
            "xT": _bf16(x[c * BC : (c + 1) * BC].reshape(TOK, D).T),
            "w_qkv": w_qkv,
            "w_out": w_out,
            "b_out": b_out,
        }
        for c in range(N_CORES)
    ]


def run_sharded(inputs, **kw):
    nc = _get_nc()
    in_maps = make_in_maps(inputs)
    res = run_bass_kernel_spmd(nc, in_maps, core_ids=list(range(N_CORES)), **kw)
    out = np.concatenate(
        [r["out"].reshape(BC, SEQ, D) for r in res.results], axis=0
    )
    return out, res


def kernel(x, w_qkv, w_out, b_out):
    out, _ = run_sharded(
        {"x": x, "w_qkv": w_qkv, "w_out": w_out, "b_out": b_out}
    )
    return out
